# revision 10
# baseline (speedup 1.0000x reference)
"""Multi-head attention kernel for TRN2, 8 NeuronCores, head-parallel.

Full problem: Q,K,V [B=4, H=8, S=4096, D=64] fp32; out = softmax(QK^T/8) V.
Sharding: 32 (b,h) slices -> 4 per core; no cross-core communication.

Per-core algorithm (heads processed in packed pairs A/B):
  - Prologue per pair, quartered and fully overlapped with compute: DMA K+V_A
    on the sync ring, Q+V_B on the scalar ring (V interleaved after the first
    K/Q quarter); DVE casts each quarter to bf16; Qt/Kt quarter tiles
    [d(A)|d(B) on partitions, s free] built via normal bf16 matmul transposes
    (lhsT=chunk, rhs=identity, ~90ns each, keeps the PE HAM warm). Quarter
    tiles (not one big tile) so the main loop's QK only depends on the
    quarter it reads -> pair 0's main loop starts ~6us in, and pair 1's
    prologue hides inside pair 0's main loop (loads hoisted, transposes
    embedded at qb 4..7).
  - Main loop, one k-chunk (128) per step, per-head psum tiles:
      scoresT[k, q] <- two row-tiled matmuls (A rows 0:64, B rows 64:128)
      issued back-to-back so they run concurrently (~259ns/pair).
      exp with a diagonal engine split to break the single-engine exp wall
      and keep the QK->exp->PV latency chain short (FD=512 per op):
        * (kc + head) even: ScalarE ACTIVATE exact exp (scale=1/8 folded)
        * (kc + head) odd:  VectorE tensor_scalar Schraudolph exp:
          i16 = round(A*s + B) bitcast to bf16 approximates exp(s/8) with
          ~2% element error, zero-mean so softmax renormalization cancels
          the bias; each head gets 50% exact / 50% approx -> ~1e-2 rel err.
      PV: stat = [V_chunk | ones] (65 cols) so the softmax denominator
      accumulates free as row 64 of outT; accumulate over 32 chunks in
      psum [65, 512] per head.
  - Epilogue per (qb, head): outT psum -> sbuf bf16 (both copies on ScalarE),
    transpose back to [q, 65] via 4 normal bf16 matmuls vs identity (fp32
    matmuls are 2-pass with unhideable LDWEIGHTS - avoid), fast reciprocal
    of col 64, scale cols 0:64 (broadcast tensor_tensor), DMA out (sync).

PSUM budget (8 banks): per-head scores [128,512] x4 bufs = 4, PV-out A/B = 2,
transpose scratch = 1, epilogue scratch = 1 (head B's epilogue borrows a
scores buf).
"""

import numpy as np

from concourse import bacc, mybir, tile
from concourse.bass_utils import run_bass_kernel_spmd
from concourse.masks import make_identity

P = 128          # partitions
S = 4096         # sequence length
D = 64           # head dim
NH = 4           # heads per core
NC = S // P      # 32 k-chunks of 128
QB = 512         # q block (psum bank free size in fp32)
NQ = S // QB     # 8 q blocks
NQTR = 4         # DMA quarters
CPQ = NC // NQTR # chunks per quarter
SQ = S // NQTR   # seq elems per quarter
FP32 = mybir.dt.float32
BF16 = mybir.dt.bfloat16
I16 = mybir.dt.int16

N_CORES = 8
SCALE = 1.0 / np.sqrt(np.float32(D))  # 0.125

# Schraudolph exp-as-bf16-bits constants (see module docstring).
# i16 = round(EXP_A * s + EXP_B); bits -> bf16 ~= exp(s * SCALE).
# EXP_A = 128 * log2(e) * SCALE; EXP_B = 128*127 + 128*c0 with c0 chosen so
# the piecewise-linear relative error is zero-mean over f ~ U[0,1).
EXP_A = float(128 * np.log2(np.e) * SCALE)
EXP_B = 16248.7807254998


def build():
    nc = bacc.Bacc("TRN2", target_bir_lowering=False)
    q_d = nc.dram_tensor("Q", (NH, S, D), FP32, kind="ExternalInput")
    k_d = nc.dram_tensor("K", (NH, S, D), FP32, kind="ExternalInput")
    v_d = nc.dram_tensor("V", (NH, S, D), FP32, kind="ExternalInput")
    o_d = nc.dram_tensor("out", (NH, S, D), FP32, kind="ExternalOutput")

    with tile.TileContext(nc) as tc:
        with (
            tc.tile_pool(name="const", bufs=1) as const_pool,
            tc.tile_pool(name="stage", bufs=4) as stage_pool,
            tc.tile_pool(name="stgb", bufs=4) as stgb_pool,
            tc.tile_pool(name="qt", bufs=2) as qt_pool,
            tc.tile_pool(name="kt", bufs=2) as kt_pool,
            tc.tile_pool(name="vsb", bufs=2) as vsb_pool,
            tc.tile_pool(name="pt", bufs=4) as pt_pool,
            tc.tile_pool(name="osb", bufs=2) as osb_pool,
            tc.tile_pool(name="fin", bufs=3) as fin_pool,
            tc.tile_pool(name="recip", bufs=3) as recip_pool,
            tc.tile_pool(name="sc", bufs=4, space="PSUM") as sc_pool,
            tc.tile_pool(name="pso_a", bufs=1, space="PSUM") as pso_a_pool,
            tc.tile_pool(name="pso_b", bufs=1, space="PSUM") as pso_b_pool,
            tc.tile_pool(name="ps_tr", bufs=1, space="PSUM") as ps_tr_pool,
            tc.tile_pool(name="ps_ep", bufs=1, space="PSUM") as ps_ep_pool,
        ):
            ident = const_pool.tile([P, P], BF16)
            make_identity(nc, ident)

            # preload the exp table-set (~2.7us) before any data arrives
            tl_src = const_pool.tile([P, 1], FP32)
            nc.vector.memset(tl_src, 0.0)
            tl_dst = const_pool.tile([P, 1], FP32)
            nc.scalar.activation(
                tl_dst, tl_src, mybir.ActivationFunctionType.Exp, scale=1.0
            )

            def load_quarter(pair, st, g, q_eng, kq_rearr):
                lo = g * CPQ
                kf = stage_pool.tile(
                    [P, CPQ, 2, D], FP32, tag="kf", name=f"kf_{pair}_{g}"
                )
                qf = stage_pool.tile(
                    [P, CPQ, 2, D], FP32, tag="qf", name=f"qf_{pair}_{g}"
                )
                vf = stage_pool.tile(
                    [P, CPQ, 2, D], FP32, tag="vf", name=f"vf_{pair}_{g}"
                )
                for h_i in range(2):
                    kr, qr, vr = kq_rearr[h_i]
                    nc.sync.dma_start(
                        out=kf[:, :, h_i, :], in_=kr[:, lo : lo + CPQ, :]
                    )
                    q_eng.dma_start(
                        out=qf[:, :, h_i, :], in_=qr[:, lo : lo + CPQ, :]
                    )
                nc.sync.dma_start(
                    out=vf[:, :, 0, :], in_=kq_rearr[0][2][:, lo : lo + CPQ, :]
                )
                q_eng.dma_start(
                    out=vf[:, :, 1, :], in_=kq_rearr[1][2][:, lo : lo + CPQ, :]
                )
                st["kf"].append(kf)
                st["qf"].append(qf)
                st["vf"].append(vf)

            def load_pair(pair):
                """Issue all DMA loads for a pair; allocate its big tiles.

                K + V_A ride the sync ring, Q + V_B the scalar ring, with the
                first V quarter right after the first K/Q quarter so the
                first PV chunk's data lands early.
                """
                ha, hb = 2 * pair, 2 * pair + 1
                st = {
                    "qt": [
                        qt_pool.tile([P, SQ], BF16, name=f"qt_{pair}_{g}", tag=f"qt{g}")
                        for g in range(NQTR)
                    ],
                    "kt": [
                        kt_pool.tile([P, SQ], BF16, name=f"kt_{pair}_{g}", tag=f"kt{g}")
                        for g in range(NQTR)
                    ],
                    "vsb": [
                        vsb_pool.tile(
                            [P, 2, CPQ, D + 1], BF16,
                            name=f"vsb_{pair}_{g}", tag=f"vsb{g}",
                        )
                        for g in range(NQTR)
                    ],
                    "kf": [], "qf": [], "vf": [],
                    "heads": (ha, hb),
                }
                for g in range(NQTR):
                    nc.gpsimd.memset(st["vsb"][g][:, :, :, D : D + 1], 1.0)
                kq_rearr = [
                    (k_d[h].rearrange("(c p) d -> p c d", p=P),
                     q_d[h].rearrange("(c p) d -> p c d", p=P),
                     v_d[h].rearrange("(c p) d -> p c d", p=P))
                    for h in (ha, hb)
                ]
                # pair 0: loads woven quarter-by-quarter with prologue
                # compute (issued by the caller); Q/V_B on the idle-at-start
                # scalar queue. Later pairs: everything up-front on the sync
                # queue so the scalar engine is never taxed with DMA issue
                # during the main loop (pair p+1's 6MB fit easily within
                # pair p's ~250us main loop).
                st["q_eng"] = nc.scalar if pair == 0 else nc.sync
                st["kq_rearr"] = kq_rearr
                if pair != 0:
                    for g in range(NQTR):
                        load_quarter(pair, st, g, st["q_eng"], kq_rearr)
                return st

            def prologue_quarter(st, g, fast):
                """bf16 casts + transposes + vsb copy for quarter g.

                fast=True (pair 0 critical path): alternate two psum scratch
                tags for 2-deep pipelining. fast=False (background while the
                previous pair computes): single tag, trickles into idle slots.
                """
                kf, qf, vf = st["kf"][g], st["qf"][g], st["vf"][g]
                kb = stgb_pool.tile(
                    [P, CPQ, 2, D], BF16, tag="kb", name=f"kb_{id(st)}_{g}"
                )
                qb2 = stgb_pool.tile(
                    [P, CPQ, 2, D], BF16, tag="qb", name=f"qb_{id(st)}_{g}"
                )
                half = CPQ // 2
                for hh in range(2):
                    s = slice(hh * half, (hh + 1) * half)
                    nc.vector.tensor_copy(kb[:, s, :, :], kf[:, s, :, :])
                    nc.vector.tensor_copy(qb2[:, s, :, :], qf[:, s, :, :])
                for c in range(CPQ):
                    for t_i, (src, dst) in enumerate(
                        ((kb, st["kt"][g]), (qb2, st["qt"][g]))
                    ):
                        if fast:
                            tag, pool = (
                                ("ps_tr", ps_tr_pool)
                                if (2 * c + t_i) % 2 == 0
                                else ("ps_ep", ps_ep_pool)
                            )
                        else:
                            tag, pool = "ps_tr", ps_tr_pool
                        ps_t = pool.tile([P, P], FP32, tag=tag)
                        nc.tensor.matmul(
                            ps_t,
                            lhsT=src[:, c, :, :].rearrange("p a b -> p (a b)"),
                            rhs=ident,
                            start=True,
                            stop=True,
                        )
                        col = dst[:, c * P : (c + 1) * P]
                        if t_i == 0:
                            nc.scalar.copy(col, ps_t)
                        else:
                            nc.vector.tensor_copy(col, ps_t)
                half = CPQ // 2
                for h_i in range(2):
                    for hh in range(2):
                        s = slice(hh * half, (hh + 1) * half)
                        nc.vector.tensor_copy(
                            st["vsb"][g][:, h_i, s, 0:D], vf[:, s, h_i, :]
                        )

            def main_pair(st, next_st):
                ha, hb = st["heads"]
                for qb in range(NQ):
                    q0 = (qb % 2) * QB
                    qt_q = st["qt"][qb // 2]
                    out_ta = pso_a_pool.tile([D + 1, QB], FP32)
                    out_tb = pso_b_pool.tile([D + 1, QB], FP32)
                    for kc in range(NC):
                        g, cq = kc // CPQ, kc % CPQ
                        kt_q = st["kt"][g]
                        sca = sc_pool.tile([P, QB], FP32, tag="sc")
                        scb = sc_pool.tile([P, QB], FP32, tag="sc")
                        nc.tensor.matmul(
                            sca,
                            lhsT=kt_q[0:64, cq * P : (cq + 1) * P],
                            rhs=qt_q[0:64, q0 : q0 + QB],
                            start=True,
                            stop=True,
                        )
                        nc.tensor.matmul(
                            scb,
                            lhsT=kt_q[64:128, cq * P : (cq + 1) * P],
                            rhs=qt_q[64:128, q0 : q0 + QB],
                            start=True,
                            stop=True,
                        )
                        pts = []
                        for h_i, sc in enumerate((sca, scb)):
                            if (kc + h_i) % 2 == 0:
                                pt = pt_pool.tile([P, QB], BF16, tag=f"pt{h_i}")
                                nc.scalar.activation(
                                    pt,
                                    sc,
                                    mybir.ActivationFunctionType.Exp,
                                    scale=SCALE,
                                )
                                pts.append(pt)
                            else:
                                pt_i = pt_pool.tile([P, QB], I16, tag=f"pt{h_i}")
                                nc.vector.tensor_scalar(
                                    out=pt_i,
                                    in0=sc,
                                    scalar1=EXP_A,
                                    scalar2=EXP_B,
                                    op0=mybir.AluOpType.mult,
                                    op1=mybir.AluOpType.add,
                                )
                                pts.append(pt_i.bitcast(BF16))
                        first = kc == 0
                        last = kc == NC - 1
                        nc.tensor.matmul(
                            out_ta,
                            lhsT=st["vsb"][g][:, 0, cq, :],
                            rhs=pts[0],
                            start=first,
                            stop=last,
                        )
                        nc.tensor.matmul(
                            out_tb,
                            lhsT=st["vsb"][g][:, 1, cq, :],
                            rhs=pts[1],
                            start=first,
                            stop=last,
                        )

                    # ---- epilogue: transpose back, normalize, store ----
                    qd = qb * QB
                    for h_i, (h, out_t) in enumerate(((ha, out_ta), (hb, out_tb))):
                        osb = osb_pool.tile([D + 1, QB], BF16)
                        nc.scalar.copy(osb, out_t)
                        if h_i == 0:
                            ps4 = ps_ep_pool.tile(
                                [P, QB // P, D + 1], FP32, tag="ps_ep"
                            )
                        else:
                            ps4 = sc_pool.tile([P, QB // P, D + 1], FP32, tag="sc")
                        for j in range(QB // P):
                            nc.tensor.matmul(
                                ps4[:, j, :],
                                lhsT=osb[:, j * P : (j + 1) * P],
                                rhs=ident[0 : D + 1, 0 : D + 1],
                                start=True,
                                stop=True,
                            )
                        rec = recip_pool.tile([P, QB // P, 1], FP32)
                        nc.vector.reciprocal_approx_fast(rec, ps4[:, :, D : D + 1])
                        fin = fin_pool.tile([P, QB // P, D], FP32)
                        nc.vector.tensor_tensor(
                            fin,
                            ps4[:, :, 0:D],
                            rec.broadcast_to((P, QB // P, D)),
                            mybir.AluOpType.mult,
                        )
                        nc.sync.dma_start(
                            out=o_d[h, qd : qd + QB, :].rearrange(
                                "(j p) d -> p j d", p=P
                            ),
                            in_=fin,
                        )

                    # background prologue for the next pair during qb 4..7
                    if next_st is not None and qb >= NQ - NQTR:
                        prologue_quarter(next_st, qb - (NQ - NQTR), fast=False)

            st0 = load_pair(0)
            for g in range(NQTR):
                load_quarter(0, st0, g, st0["q_eng"], st0["kq_rearr"])
                prologue_quarter(st0, g, fast=True)
            st1 = load_pair(1)
            main_pair(st0, st1)
            main_pair(st1, None)

    nc.compile()
    return nc


_NC_CACHE = None


def _get_nc():
    global _NC_CACHE
    if _NC_CACHE is None:
        _NC_CACHE = build()
    return _NC_CACHE


def kernel(Q, K, V):
    Q = np.ascontiguousarray(np.asarray(Q, dtype=np.float32))
    K = np.ascontiguousarray(np.asarray(K, dtype=np.float32))
    V = np.ascontiguousarray(np.asarray(V, dtype=np.float32))
    B, H = Q.shape[0], Q.shape[1]
    qr = Q.reshape(B * H, S, D)
    kr = K.reshape(B * H, S, D)
    vr = V.reshape(B * H, S, D)
    in_maps = [
        {
            "Q": qr[i * NH : (i + 1) * NH],
            "K": kr[i * NH : (i + 1) * NH],
            "V": vr[i * NH : (i + 1) * NH],
        }
        for i in range(N_CORES)
    ]
    res = run_bass_kernel_spmd(_get_nc(), in_maps, core_ids=list(range(N_CORES)))
    out = np.concatenate([res.results[i]["out"] for i in range(N_CORES)], axis=0)
    return out.reshape(B, H, S, D)
